# revision 44
# baseline (speedup 1.0000x reference)
"""Causal self-attention Trainium2 kernel (Bass/Tile), batch-data-parallel over 8 cores.

Per core (batch element): x [2048, 768] -> y [2048, 768], k/v [12, 2048, 64].
All matmuls in bf16 (fp32 accumulation in PSUM). Softmax without max-subtraction
(scores are bounded ~|2.5| at this problem's scale); the denominator comes from a
ones-column appended to V inside the attention @ V matmul.

Software-pipelined over t-chunks of 512: while attention of chunk i runs (ScalarE
exp stream is the bottleneck there), the PE work of chunk i+1's QKV phase and
chunk i-1's output projection is interleaved into the PE instruction stream.
"""

from contextlib import ExitStack
from itertools import chain

import numpy as np

import concourse.bacc as bacc
import concourse.bass as bass
import concourse.mybir as mybir
import concourse.tile as tile
from concourse.bass_utils import run_bass_kernel_spmd
from concourse.masks import make_identity

B, T, C, H = 8, 2048, 768, 12
D = C // H          # 64
P = 128
TC = C // P         # 6 chunks of C
NT = T // P         # 16 t-tiles
TCH = T // 512      # 4 t-chunks of 512
NPAIR = H // 2      # 6 head pairs
F32 = mybir.dt.float32
BF = mybir.dt.bfloat16
ADD = mybir.AluOpType.add
MUL = mybir.AluOpType.mult
EXP = mybir.ActivationFunctionType.Exp


def _row_bcast(ap, offset, rows, n):
    """DRAM row [n] broadcast to [rows, n] (partition-step-0 source AP)."""
    return bass.AP(tensor=ap.tensor, offset=ap.offset + offset,
                   ap=[[0, rows], [1, n]])


def build_nc():
    nc = bacc.Bacc(trn_type="TRN2", target_bir_lowering=False)

    x = nc.dram_tensor("x", [T, C], F32, kind="ExternalInput")[:]
    w_attn = nc.dram_tensor("w_attn", [C, 3 * C], F32, kind="ExternalInput")[:]
    b_attn = nc.dram_tensor("b_attn", [3 * C], F32, kind="ExternalInput")[:]
    w_proj = nc.dram_tensor("w_proj", [C, C], F32, kind="ExternalInput")[:]
    b_proj = nc.dram_tensor("b_proj", [C], F32, kind="ExternalInput")[:]
    y = nc.dram_tensor("y", [T, C], F32, kind="ExternalOutput")[:]
    k_out = nc.dram_tensor("k_out", [H, T, D], F32, kind="ExternalOutput")[:]
    v_out = nc.dram_tensor("v_out", [H, T, D], F32, kind="ExternalOutput")[:]

    with tile.TileContext(nc) as tc:
        _body(tc, x, w_attn, b_attn, w_proj, b_proj, y, k_out, v_out)
    nc.compile()
    return nc


def _body(tc, x, w_attn, b_attn, w_proj, b_proj, y, k_out, v_out):
    nc = tc.nc

    with ExitStack() as ctx:
        fixed = ctx.enter_context(tc.tile_pool(name="fixed", bufs=1))
        roll = ctx.enter_context(tc.tile_pool(name="roll", bufs=2))
        epool = ctx.enter_context(tc.tile_pool(name="epool", bufs=6))
        dpool = ctx.enter_context(tc.tile_pool(name="dpool", bufs=2, space="DRAM"))
        psmm = ctx.enter_context(tc.tile_pool(name="psmm", bufs=2, space="PSUM"))
        pss = ctx.enter_context(tc.tile_pool(name="pss", bufs=2, space="PSUM"))
        psy = ctx.enter_context(tc.tile_pool(name="psy", bufs=2, space="PSUM"))

        # ---- constants (tiles now; instructions emitted in setup_consts) ----
        ident = fixed.tile([P, P], BF, name="ident")
        tri = fixed.tile([P, P], BF, name="tri")
        b_qk = fixed.tile([P, 2 * TC], F32, name="b_qk")
        b_kv_bc = fixed.tile([P, 2 * C], F32, name="b_kv_bc")
        b_pj_bc = fixed.tile([P, C], F32, name="b_pj_bc")

        def setup_consts():
            make_identity(nc, ident)
            # tri[s, t] = 1 if t >= s else 0 (diagonal 128x128 block mask)
            nc.gpsimd.memset(tri, 1.0)
            nc.gpsimd.affine_select(
                out=tri, in_=tri, compare_op=mybir.AluOpType.is_ge,
                fill=0.0, base=0, pattern=[[1, P]], channel_multiplier=-1)
            nc.sync.dma_start(b_qk,
                              b_attn[0:2 * C].rearrange("(j p) -> p j", p=P))
            nc.sync.dma_start(b_kv_bc, _row_bcast(b_attn, C, P, 2 * C))
            nc.sync.dma_start(b_pj_bc, _row_bcast(b_proj, 0, P, C))

        w_bf = fixed.tile([P, TC, 3 * C], BF, name="w_bf")
        wp_bf = fixed.tile([P, TC, C], BF, name="wp_bf")
        kT_bf = fixed.tile([P, TC, T], BF, name="kT_bf")
        v_aug = fixed.tile([P, NT, H, D + 1], BF, name="v_aug")
        nc.vector.memset(v_aug[:, :, :, D:D + 1], 1.0)

        xT_tiles = {}
        qT_tiles = {}
        yT_tiles = {}

        def emit_w_loads(piece):
            """Load + cast one 768-column piece of w_attn (all row chunks);
            piece 3 loads w_proj. Piece-major order lets the chunk-0 qkT
            matmuls start after only piece 0 is resident."""
            for j in range(TC):
                ws = roll.tile([P, C], F32, name="ws", tag="ws", bufs=4)
                if piece < 3:
                    nc.sync.dma_start(
                        ws, w_attn[j * P:(j + 1) * P,
                                   piece * C:(piece + 1) * C])
                    dst = w_bf[:, j, piece * C:(piece + 1) * C]
                else:
                    nc.sync.dma_start(ws, w_proj[j * P:(j + 1) * P, :])
                    dst = wp_bf[:, j, :]
                # prologue-only: ACT is idle here, split the cast load
                if j % 2:
                    nc.scalar.copy(out=dst, in_=ws)
                else:
                    nc.vector.tensor_copy(out=dst, in_=ws)
                yield

        def emit_p0(tch):
            """x load + transpose for t-chunk `tch` (generator)."""
            xT_bf = roll.tile([P, TC, 512], BF, name="xT_bf", tag="xT")
            xT_tiles[tch] = xT_bf
            x_sbs = []
            for ii in range(4):
                i = 4 * tch + ii
                x_sb = roll.tile([P, C], F32, name="x_sb", tag="x_sb",
                                 bufs=4)
                nc.sync.dma_start(x_sb, x[i * P:(i + 1) * P, :])
                x_sbs.append(x_sb)
            yield
            for ii in range(4):
                x_bf = roll.tile([P, C], BF, name="x_bf", tag="x_bf")
                nc.vector.tensor_copy(out=x_bf, in_=x_sbs[ii])
                yield
                for j in range(TC):
                    pst = psmm.tile([P, P], BF, name="pst", tag="mm512")
                    nc.tensor.transpose(pst, x_bf[:, j * P:(j + 1) * P], ident)
                    nc.vector.tensor_copy(
                        out=xT_bf[:, j, ii * P:(ii + 1) * P], in_=pst)
                    if j % 2:
                        yield

        def emit_a1(tch, jcs):
            """qkT matmuls for t-chunk `tch` over c'-tiles `jcs` (generator)."""
            tsl = bass.ds(tch * 512, 512)
            xT_bf = xT_tiles[tch]
            if tch not in qT_tiles:
                qT_tiles[tch] = roll.tile([P, TC, 512], BF, name="qT_bf",
                                          tag="qT")
            qT_bf = qT_tiles[tch]
            for jc in jcs:
                ps = psmm.tile([P, 512], F32, name="ps_qk", tag="mm512")
                for j in range(TC):
                    nc.tensor.matmul(
                        ps, lhsT=w_bf[:, j, jc * P:(jc + 1) * P],
                        rhs=xT_bf[:, j, :],
                        start=(j == 0), stop=(j == TC - 1))
                dst = (qT_bf[:, jc, :] if jc < TC
                       else kT_bf[:, jc - TC, tsl])
                nc.vector.tensor_scalar_add(
                    out=dst, in0=ps, scalar1=b_qk[:, jc:jc + 1])
                yield

        def emit_a2(tch):
            """v natural-layout matmuls + k/v outputs for t-chunk `tch`.
            k_out comes from transposing kT_bf (bias already included) —
            saves the 768-column k-natural matmul on the PE."""
            xT_bf = xT_tiles[tch]
            tsl = bass.ds(tch * 512, 512)
            for ii in range(4):
                i = 4 * tch + ii
                tslice = bass.ds(i * P, P)
                # k_out: PE-transpose kT tiles back to [t, d] and cast to f32
                kst = roll.tile([P, C], F32, name="kst", tag="kst")
                for j in range(TC):
                    pst = psmm.tile([P, P], BF, name="pstk", tag="mm512")
                    nc.tensor.transpose(
                        pst, kT_bf[:, j, i * P:(i + 1) * P], ident)
                    nc.vector.tensor_copy(
                        out=kst[:, j * P:(j + 1) * P], in_=pst)
                    if j % 2:
                        yield
                nc.sync.dma_start(
                    out=k_out[:, tslice, :].rearrange("h p d -> p h d"),
                    in_=kst.rearrange("p (h d) -> p h d", h=H))
                # v natural via matmul
                vf = roll.tile([P, C], F32, name="vf", tag="kvf", bufs=2)
                for nb, nw in ((0, 512), (1, 256)):
                    ps = psmm.tile([P, 512], F32, name="ps_kv", tag="mm512")
                    c0 = 2 * C + nb * 512
                    for j in range(TC):
                        nc.tensor.matmul(
                            ps[:, 0:nw], lhsT=xT_bf[:, j, ii * P:(ii + 1) * P],
                            rhs=w_bf[:, j, c0:c0 + nw],
                            start=(j == 0), stop=(j == TC - 1))
                    nc.vector.tensor_tensor(
                        out=vf[:, nb * 512:nb * 512 + nw], in0=ps[:, 0:nw],
                        in1=b_kv_bc[:, C + nb * 512:C + nb * 512 + nw], op=ADD)
                    yield
                nc.sync.dma_start(
                    out=v_out[:, tslice, :].rearrange("h p d -> p h d"),
                    in_=vf.rearrange("p (h d) -> p h d", h=H))
                nc.vector.tensor_copy(
                    out=v_aug[:, i, :, 0:D],
                    in_=vf.rearrange("p (h d) -> p h d", h=H))
                yield

        def emit_c(tch):
            """Output projection for t-chunk `tch` (generator)."""
            yT_bf = yT_tiles[tch]
            for ii in range(4):
                i = 4 * tch + ii
                isl = bass.ds(ii * P, P)
                pc0 = psmm.tile([P, 512], F32, name="pc0", tag="mm512")
                for j in range(TC):
                    nc.tensor.matmul(pc0, lhsT=yT_bf[:, j, isl],
                                     rhs=wp_bf[:, j, 0:512],
                                     start=(j == 0), stop=(j == TC - 1))
                yield
                pc1 = psmm.tile([P, 256], F32, name="pc1", tag="mm512")
                for j in range(TC):
                    nc.tensor.matmul(pc1, lhsT=yT_bf[:, j, isl],
                                     rhs=wp_bf[:, j, 512:768],
                                     start=(j == 0), stop=(j == TC - 1))
                yield
                y_sb = roll.tile([P, C], F32, name="y_sb", tag="y_sb")
                nc.vector.tensor_tensor(out=y_sb[:, 0:512], in0=pc0,
                                        in1=b_pj_bc[:, 0:512], op=ADD)
                nc.vector.tensor_tensor(out=y_sb[:, 512:768], in0=pc1,
                                        in1=b_pj_bc[:, 512:768], op=ADD)
                nc.sync.dma_start(out=y[i * P:(i + 1) * P, :], in_=y_sb)
                yield

        def emit_b(tch, feeder, n_units):
            """Attention for t-chunk `tch`, pulling PE filler work from
            `feeder` between the score and AV stages. The `n_units` filler
            items are spread evenly over all pull points."""
            n_act_ = 4 * (tch + 1)
            points = NPAIR * (n_act_ + 4)
            state = {"point": 0, "consumed": 0}

            def pull(_k=None):
                state["point"] += 1
                target = min(n_units, (state["point"] * n_units + points - 1)
                             // points)
                while state["consumed"] < target:
                    if next(feeder, _SENT) is _SENT:
                        state["consumed"] = n_units
                        return
                    state["consumed"] += 1

            n_act = 4 * (tch + 1)
            n_full = 4 * tch
            yT_bf = roll.tile([P, TC, 512], BF, name="yT_bf", tag="yT")
            yT_tiles[tch] = yT_bf
            qT_bf = qT_tiles[tch]


            for p in range(NPAIR):
                yA = psy.tile([D + 1, 512], F32, name="yA", tag="y")
                yB = psy.tile([D + 1, 512], F32, name="yB", tag="y")
                e_tiles = {}

                def av(st):
                    eAB = e_tiles.pop(st)
                    first = st == 0
                    last = st == n_act - 1
                    if st < n_full:
                        nc.tensor.matmul(
                            yA, lhsT=v_aug[:, st, 2 * p, :],
                            rhs=eAB[:, 0:512], start=first, stop=False)
                        nc.tensor.matmul(
                            yB, lhsT=v_aug[:, st, 2 * p + 1, :],
                            rhs=eAB[:, 512:1024], start=first, stop=False)
                    else:
                        m = st - 4 * tch
                        for q in range(m, 4):
                            qsl = bass.ds(q * P, P)
                            qsl2 = bass.ds(512 + q * P, P)
                            stp = last and q == 3
                            nc.tensor.matmul(
                                yA[:, qsl], lhsT=v_aug[:, st, 2 * p, :],
                                rhs=eAB[:, qsl],
                                start=(first and q == m), stop=stp)
                            nc.tensor.matmul(
                                yB[:, qsl], lhsT=v_aug[:, st, 2 * p + 1, :],
                                rhs=eAB[:, qsl2],
                                start=(first and q == m), stop=stp)

                for st in range(n_act):
                    ssl = bass.ds(st * P, P)
                    psAB = pss.tile([P, 1024], F32, name="psAB", tag="s")
                    nc.tensor.matmul(
                        psAB[:, 0:512], lhsT=kT_bf[0:D, p, ssl],
                        rhs=qT_bf[0:D, p, :], start=True, stop=True)
                    nc.tensor.matmul(
                        psAB[:, 512:1024], lhsT=kT_bf[D:P, p, ssl],
                        rhs=qT_bf[D:P, p, :], start=True, stop=True)
                    eAB = epool.tile([P, 1024], BF, name="eAB", tag="eAB")
                    e_tiles[st] = eAB
                    if st < n_full:
                        nc.scalar.activation(out=eAB, in_=psAB, func=EXP,
                                             scale=0.125)
                    else:
                        r = (st - 4 * tch) * P
                        nc.scalar.activation(out=eAB[:, r:1024],
                                             in_=psAB[:, r:1024],
                                             func=EXP, scale=0.125)
                        # mask the two diagonal 128x128 blocks (heads A+B)
                        nc.vector.tensor_tensor(
                            out=eAB[:, r:r + P], in0=eAB[:, r:r + P],
                            in1=tri, op=MUL)
                        nc.vector.tensor_tensor(
                            out=eAB[:, 512 + r:512 + r + P],
                            in0=eAB[:, 512 + r:512 + r + P],
                            in1=tri, op=MUL)
                    pull(2)
                    if st >= 2:
                        av(st - 2)
                pull(2)
                av(n_act - 2)
                pull(2)
                av(n_act - 1)

                # normalize: row D holds sum(exp); broadcast 1/sum across
                # partitions via a DRAM roundtrip (engines cannot move data
                # across partitions)
                yAf = roll.tile([D + 1, 512], F32, name="yAf", tag="yAf")
                yBf = roll.tile([D + 1, 512], F32, name="yBf", tag="yBf")
                nc.vector.tensor_copy(out=yAf, in_=yA)
                nc.vector.tensor_copy(out=yBf, in_=yB)
                nc.vector.reciprocal(out=yAf[D:D + 1, :], in_=yAf[D:D + 1, :])
                nc.vector.reciprocal(out=yBf[D:D + 1, :], in_=yBf[D:D + 1, :])
                rc_dram = dpool.tile([2, 512], F32, name="rc_dram",
                                     tag="rc_dram")
                nc.sync.dma_start(rc_dram[0:1, :], yAf[D:D + 1, :])
                nc.sync.dma_start(rc_dram[1:2, :], yBf[D:D + 1, :])
                pull(2)
                bc = roll.tile([D, 2, 512], F32, name="bc", tag="bc")
                nc.sync.dma_start(
                    bc, bass.AP(tensor=rc_dram.tensor, offset=rc_dram.offset,
                                ap=[[0, D], [512, 2], [1, 512]]))
                nc.vector.tensor_tensor(
                    out=yT_bf[0:D, p, :], in0=yAf[0:D, :], in1=bc[:, 0, :],
                    op=MUL)
                stB = roll.tile([D, 512], BF, name="stB", tag="stB")
                nc.vector.tensor_tensor(
                    out=stB, in0=yBf[0:D, :], in1=bc[:, 1, :], op=MUL)
                nc.sync.dma_start(out=yT_bf[D:P, p, :], in_=stB)
                pull(2)


        _SENT = object()

        # prologue: x DMAs first, then constants, then x pipeline of chunk 0
        # interleaved with weight piece 0, then chunk-0 qkT/kv matmuls
        # chasing the remaining weight pieces
        p00 = emit_p0(0)
        next(p00, None)     # issue the 4 x DMAs of chunk 0
        setup_consts()
        for _ in emit_w_loads(0):
            next(p00, None)
            next(p00, None)
        for _ in p00:
            pass
        g = emit_a1(0, range(TC))
        for _ in emit_w_loads(1):
            next(g, None)
        for _ in g:
            pass
        g = emit_a1(0, range(TC, 2 * TC))
        for _ in emit_w_loads(2):
            next(g, None)
        for _ in g:
            pass
        g = emit_a2(0)
        for _ in emit_w_loads(3):
            next(g, None)
        for _ in g:
            pass

        for tch in range(TCH):
            parts = []
            n_units = 0
            if tch >= 1:
                parts.append(emit_c(tch - 1))
                n_units += 12
            if tch + 1 < TCH:
                parts.append(emit_p0(tch + 1))
                parts.append(emit_a1(tch + 1, range(2 * TC)))
                parts.append(emit_a2(tch + 1))
                n_units += 17 + 12 + 24
            feeder = chain(*parts)
            emit_b(tch, feeder, n_units)
            for _ in feeder:   # drain any remaining filler work
                pass
        for _ in emit_c(TCH - 1):
            pass


_NC_CACHE = None


def _get_nc():
    global _NC_CACHE
    if _NC_CACHE is None:
        _NC_CACHE = build_nc()
    return _NC_CACHE


def _run(inputs, trace=False):
    nc = _get_nc()
    x = np.ascontiguousarray(np.asarray(inputs["x"], dtype=np.float32))
    w_attn = np.ascontiguousarray(np.asarray(inputs["w_attn"], dtype=np.float32))
    b_attn = np.ascontiguousarray(np.asarray(inputs["b_attn"], dtype=np.float32))
    w_proj = np.ascontiguousarray(np.asarray(inputs["w_proj"], dtype=np.float32))
    b_proj = np.ascontiguousarray(np.asarray(inputs["b_proj"], dtype=np.float32))
    in_maps = [
        {"x": x[i], "w_attn": w_attn, "b_attn": b_attn,
         "w_proj": w_proj, "b_proj": b_proj}
        for i in range(B)
    ]
    res = run_bass_kernel_spmd(nc, in_maps, core_ids=list(range(B)),
                               trace=trace)
    yo = np.stack([r["y"] for r in res.results])
    ko = np.stack([r["k_out"] for r in res.results])
    vo = np.stack([r["v_out"] for r in res.results])
    return (yo, ko, vo), res


def kernel(**inputs):
    outs, _ = _run(inputs, trace=False)
    return outs


def kernel_traced(**inputs):
    outs, res = _run(inputs, trace=True)
    return outs, res


# revision 45
# speedup vs baseline: 1.0096x; 1.0096x over previous
"""Causal self-attention Trainium2 kernel (Bass/Tile), batch-data-parallel over 8 cores.

Per core (batch element): x [2048, 768] -> y [2048, 768], k/v [12, 2048, 64].
All matmuls in bf16 (fp32 accumulation in PSUM). Softmax without max-subtraction
(scores are bounded ~|2.5| at this problem's scale); the denominator comes from a
ones-column appended to V inside the attention @ V matmul.

Software-pipelined over t-chunks of 512: while attention of chunk i runs (ScalarE
exp stream is the bottleneck there), the PE work of chunk i+1's QKV phase and
chunk i-1's output projection is interleaved into the PE instruction stream.
"""

from contextlib import ExitStack
from itertools import chain

import numpy as np

import concourse.bacc as bacc
import concourse.bass as bass
import concourse.mybir as mybir
import concourse.tile as tile
from concourse.bass_utils import run_bass_kernel_spmd
from concourse.masks import make_identity

B, T, C, H = 8, 2048, 768, 12
D = C // H          # 64
P = 128
TC = C // P         # 6 chunks of C
NT = T // P         # 16 t-tiles
TCH = T // 512      # 4 t-chunks of 512
NPAIR = H // 2      # 6 head pairs
F32 = mybir.dt.float32
BF = mybir.dt.bfloat16
ADD = mybir.AluOpType.add
MUL = mybir.AluOpType.mult
EXP = mybir.ActivationFunctionType.Exp


def _row_bcast(ap, offset, rows, n):
    """DRAM row [n] broadcast to [rows, n] (partition-step-0 source AP)."""
    return bass.AP(tensor=ap.tensor, offset=ap.offset + offset,
                   ap=[[0, rows], [1, n]])


def build_nc():
    nc = bacc.Bacc(trn_type="TRN2", target_bir_lowering=False)

    x = nc.dram_tensor("x", [T, C], F32, kind="ExternalInput")[:]
    w_attn = nc.dram_tensor("w_attn", [C, 3 * C], F32, kind="ExternalInput")[:]
    b_attn = nc.dram_tensor("b_attn", [3 * C], F32, kind="ExternalInput")[:]
    w_proj = nc.dram_tensor("w_proj", [C, C], F32, kind="ExternalInput")[:]
    b_proj = nc.dram_tensor("b_proj", [C], F32, kind="ExternalInput")[:]
    y = nc.dram_tensor("y", [T, C], F32, kind="ExternalOutput")[:]
    k_out = nc.dram_tensor("k_out", [H, T, D], F32, kind="ExternalOutput")[:]
    v_out = nc.dram_tensor("v_out", [H, T, D], F32, kind="ExternalOutput")[:]

    with tile.TileContext(nc) as tc:
        _body(tc, x, w_attn, b_attn, w_proj, b_proj, y, k_out, v_out)
    nc.compile()
    return nc


def _body(tc, x, w_attn, b_attn, w_proj, b_proj, y, k_out, v_out):
    nc = tc.nc

    with ExitStack() as ctx:
        fixed = ctx.enter_context(tc.tile_pool(name="fixed", bufs=1))
        roll = ctx.enter_context(tc.tile_pool(name="roll", bufs=2))
        epool = ctx.enter_context(tc.tile_pool(name="epool", bufs=6))
        dpool = ctx.enter_context(tc.tile_pool(name="dpool", bufs=2, space="DRAM"))
        psmm = ctx.enter_context(tc.tile_pool(name="psmm", bufs=2, space="PSUM"))
        pss = ctx.enter_context(tc.tile_pool(name="pss", bufs=2, space="PSUM"))
        psy = ctx.enter_context(tc.tile_pool(name="psy", bufs=2, space="PSUM"))

        # ---- constants (tiles now; instructions emitted in setup_consts) ----
        ident = fixed.tile([P, P], BF, name="ident")
        tri = fixed.tile([P, P], BF, name="tri")
        b_qk = fixed.tile([P, 2 * TC], F32, name="b_qk")
        b_kv_bc = fixed.tile([P, 2 * C], F32, name="b_kv_bc")
        b_pj_bc = fixed.tile([P, C], F32, name="b_pj_bc")

        def setup_consts():
            make_identity(nc, ident)
            # tri[s, t] = 1 if t >= s else 0 (diagonal 128x128 block mask)
            nc.gpsimd.memset(tri, 1.0)
            nc.gpsimd.affine_select(
                out=tri, in_=tri, compare_op=mybir.AluOpType.is_ge,
                fill=0.0, base=0, pattern=[[1, P]], channel_multiplier=-1)
            nc.sync.dma_start(b_qk,
                              b_attn[0:2 * C].rearrange("(j p) -> p j", p=P))
            nc.sync.dma_start(b_kv_bc, _row_bcast(b_attn, C, P, 2 * C))
            nc.sync.dma_start(b_pj_bc, _row_bcast(b_proj, 0, P, C))

        w_bf = fixed.tile([P, TC, 3 * C], BF, name="w_bf")
        wp_bf = fixed.tile([P, TC, C], BF, name="wp_bf")
        kT_bf = fixed.tile([P, TC, T], BF, name="kT_bf")
        v_aug = fixed.tile([P, NT, H, D + 1], BF, name="v_aug")
        nc.vector.memset(v_aug[:, :, :, D:D + 1], 1.0)

        xT_tiles = {}
        qT_tiles = {}
        yT_tiles = {}

        def emit_w_loads(piece):
            """Load + cast one 768-column piece of w_attn (all row chunks);
            piece 3 loads w_proj. Piece-major order lets the chunk-0 qkT
            matmuls start after only piece 0 is resident."""
            for j in range(TC):
                ws = roll.tile([P, C], F32, name="ws", tag="ws", bufs=4)
                if piece < 3:
                    nc.sync.dma_start(
                        ws, w_attn[j * P:(j + 1) * P,
                                   piece * C:(piece + 1) * C])
                    dst = w_bf[:, j, piece * C:(piece + 1) * C]
                else:
                    nc.sync.dma_start(ws, w_proj[j * P:(j + 1) * P, :])
                    dst = wp_bf[:, j, :]
                # prologue-only: ACT is idle here, split the cast load
                if j % 2:
                    nc.scalar.copy(out=dst, in_=ws)
                else:
                    nc.vector.tensor_copy(out=dst, in_=ws)
                yield

        def emit_p0(tch):
            """x load + transpose for t-chunk `tch` (generator)."""
            xT_bf = roll.tile([P, TC, 512], BF, name="xT_bf", tag="xT")
            xT_tiles[tch] = xT_bf
            x_sbs = []
            for ii in range(4):
                i = 4 * tch + ii
                x_sb = roll.tile([P, C], F32, name="x_sb", tag="x_sb",
                                 bufs=4)
                nc.sync.dma_start(x_sb, x[i * P:(i + 1) * P, :])
                x_sbs.append(x_sb)
            yield
            for ii in range(4):
                x_bf = roll.tile([P, C], BF, name="x_bf", tag="x_bf")
                nc.vector.tensor_copy(out=x_bf, in_=x_sbs[ii])
                yield
                for j in range(TC):
                    pst = psmm.tile([P, P], BF, name="pst", tag="mm512")
                    nc.tensor.transpose(pst, x_bf[:, j * P:(j + 1) * P], ident)
                    nc.vector.tensor_copy(
                        out=xT_bf[:, j, ii * P:(ii + 1) * P], in_=pst)
                    if j % 2:
                        yield

        def emit_a1(tch, jcs):
            """qkT matmuls for t-chunk `tch` over c'-tiles `jcs` (generator)."""
            tsl = bass.ds(tch * 512, 512)
            xT_bf = xT_tiles[tch]
            if tch not in qT_tiles:
                qT_tiles[tch] = roll.tile([P, TC, 512], BF, name="qT_bf",
                                          tag="qT")
            qT_bf = qT_tiles[tch]
            for jc in jcs:
                ps = psmm.tile([P, 512], F32, name="ps_qk", tag="mm512")
                for j in range(TC):
                    nc.tensor.matmul(
                        ps, lhsT=w_bf[:, j, jc * P:(jc + 1) * P],
                        rhs=xT_bf[:, j, :],
                        start=(j == 0), stop=(j == TC - 1))
                    if j == 2:
                        yield
                dst = (qT_bf[:, jc, :] if jc < TC
                       else kT_bf[:, jc - TC, tsl])
                nc.vector.tensor_scalar_add(
                    out=dst, in0=ps, scalar1=b_qk[:, jc:jc + 1])
                yield

        def emit_a2(tch):
            """v natural-layout matmuls + k/v outputs for t-chunk `tch`.
            k_out comes from transposing kT_bf (bias already included) —
            saves the 768-column k-natural matmul on the PE."""
            xT_bf = xT_tiles[tch]
            tsl = bass.ds(tch * 512, 512)
            for ii in range(4):
                i = 4 * tch + ii
                tslice = bass.ds(i * P, P)
                # k_out: PE-transpose kT tiles back to [t, d] and cast to f32
                kst = roll.tile([P, C], F32, name="kst", tag="kst")
                for j in range(TC):
                    pst = psmm.tile([P, P], BF, name="pstk", tag="mm512")
                    nc.tensor.transpose(
                        pst, kT_bf[:, j, i * P:(i + 1) * P], ident)
                    nc.vector.tensor_copy(
                        out=kst[:, j * P:(j + 1) * P], in_=pst)
                    if j % 2:
                        yield
                nc.sync.dma_start(
                    out=k_out[:, tslice, :].rearrange("h p d -> p h d"),
                    in_=kst.rearrange("p (h d) -> p h d", h=H))
                # v natural via matmul
                vf = roll.tile([P, C], F32, name="vf", tag="kvf", bufs=2)
                for nb, nw in ((0, 512), (1, 256)):
                    ps = psmm.tile([P, 512], F32, name="ps_kv", tag="mm512")
                    c0 = 2 * C + nb * 512
                    for j in range(TC):
                        nc.tensor.matmul(
                            ps[:, 0:nw], lhsT=xT_bf[:, j, ii * P:(ii + 1) * P],
                            rhs=w_bf[:, j, c0:c0 + nw],
                            start=(j == 0), stop=(j == TC - 1))
                        if j == 2:
                            yield
                    nc.vector.tensor_tensor(
                        out=vf[:, nb * 512:nb * 512 + nw], in0=ps[:, 0:nw],
                        in1=b_kv_bc[:, C + nb * 512:C + nb * 512 + nw], op=ADD)
                    yield
                nc.sync.dma_start(
                    out=v_out[:, tslice, :].rearrange("h p d -> p h d"),
                    in_=vf.rearrange("p (h d) -> p h d", h=H))
                nc.vector.tensor_copy(
                    out=v_aug[:, i, :, 0:D],
                    in_=vf.rearrange("p (h d) -> p h d", h=H))
                yield

        def emit_c(tch):
            """Output projection for t-chunk `tch` (generator)."""
            yT_bf = yT_tiles[tch]
            for ii in range(4):
                i = 4 * tch + ii
                isl = bass.ds(ii * P, P)
                pc0 = psmm.tile([P, 512], F32, name="pc0", tag="mm512")
                for j in range(TC):
                    nc.tensor.matmul(pc0, lhsT=yT_bf[:, j, isl],
                                     rhs=wp_bf[:, j, 0:512],
                                     start=(j == 0), stop=(j == TC - 1))
                    if j == 2:
                        yield
                yield
                pc1 = psmm.tile([P, 256], F32, name="pc1", tag="mm512")
                for j in range(TC):
                    nc.tensor.matmul(pc1, lhsT=yT_bf[:, j, isl],
                                     rhs=wp_bf[:, j, 512:768],
                                     start=(j == 0), stop=(j == TC - 1))
                    if j == 2:
                        yield
                yield
                y_sb = roll.tile([P, C], F32, name="y_sb", tag="y_sb")
                nc.vector.tensor_tensor(out=y_sb[:, 0:512], in0=pc0,
                                        in1=b_pj_bc[:, 0:512], op=ADD)
                nc.vector.tensor_tensor(out=y_sb[:, 512:768], in0=pc1,
                                        in1=b_pj_bc[:, 512:768], op=ADD)
                nc.sync.dma_start(out=y[i * P:(i + 1) * P, :], in_=y_sb)
                yield

        def emit_b(tch, feeder, n_units):
            """Attention for t-chunk `tch`, pulling PE filler work from
            `feeder` between the score and AV stages. The `n_units` filler
            items are spread evenly over all pull points."""
            n_act_ = 4 * (tch + 1)
            points = NPAIR * (n_act_ + 4)
            state = {"point": 0, "consumed": 0}

            def pull(_k=None):
                state["point"] += 1
                target = min(n_units, (state["point"] * n_units + points - 1)
                             // points)
                while state["consumed"] < target:
                    if next(feeder, _SENT) is _SENT:
                        state["consumed"] = n_units
                        return
                    state["consumed"] += 1

            n_act = 4 * (tch + 1)
            n_full = 4 * tch
            yT_bf = roll.tile([P, TC, 512], BF, name="yT_bf", tag="yT")
            yT_tiles[tch] = yT_bf
            qT_bf = qT_tiles[tch]


            for p in range(NPAIR):
                yA = psy.tile([D + 1, 512], F32, name="yA", tag="y")
                yB = psy.tile([D + 1, 512], F32, name="yB", tag="y")
                e_tiles = {}

                def av(st):
                    eAB = e_tiles.pop(st)
                    first = st == 0
                    last = st == n_act - 1
                    if st < n_full:
                        nc.tensor.matmul(
                            yA, lhsT=v_aug[:, st, 2 * p, :],
                            rhs=eAB[:, 0:512], start=first, stop=False)
                        nc.tensor.matmul(
                            yB, lhsT=v_aug[:, st, 2 * p + 1, :],
                            rhs=eAB[:, 512:1024], start=first, stop=False)
                    else:
                        m = st - 4 * tch
                        for q in range(m, 4):
                            qsl = bass.ds(q * P, P)
                            qsl2 = bass.ds(512 + q * P, P)
                            stp = last and q == 3
                            nc.tensor.matmul(
                                yA[:, qsl], lhsT=v_aug[:, st, 2 * p, :],
                                rhs=eAB[:, qsl],
                                start=(first and q == m), stop=stp)
                            nc.tensor.matmul(
                                yB[:, qsl], lhsT=v_aug[:, st, 2 * p + 1, :],
                                rhs=eAB[:, qsl2],
                                start=(first and q == m), stop=stp)

                for st in range(n_act):
                    ssl = bass.ds(st * P, P)
                    psAB = pss.tile([P, 1024], F32, name="psAB", tag="s")
                    nc.tensor.matmul(
                        psAB[:, 0:512], lhsT=kT_bf[0:D, p, ssl],
                        rhs=qT_bf[0:D, p, :], start=True, stop=True)
                    nc.tensor.matmul(
                        psAB[:, 512:1024], lhsT=kT_bf[D:P, p, ssl],
                        rhs=qT_bf[D:P, p, :], start=True, stop=True)
                    eAB = epool.tile([P, 1024], BF, name="eAB", tag="eAB")
                    e_tiles[st] = eAB
                    if st < n_full:
                        nc.scalar.activation(out=eAB, in_=psAB, func=EXP,
                                             scale=0.125)
                    else:
                        r = (st - 4 * tch) * P
                        nc.scalar.activation(out=eAB[:, r:1024],
                                             in_=psAB[:, r:1024],
                                             func=EXP, scale=0.125)
                        # mask the two diagonal 128x128 blocks (heads A+B)
                        nc.vector.tensor_tensor(
                            out=eAB[:, r:r + P], in0=eAB[:, r:r + P],
                            in1=tri, op=MUL)
                        nc.vector.tensor_tensor(
                            out=eAB[:, 512 + r:512 + r + P],
                            in0=eAB[:, 512 + r:512 + r + P],
                            in1=tri, op=MUL)
                    pull(2)
                    if st >= 2:
                        av(st - 2)
                pull(2)
                av(n_act - 2)
                pull(2)
                av(n_act - 1)

                # normalize: row D holds sum(exp); broadcast 1/sum across
                # partitions via a DRAM roundtrip (engines cannot move data
                # across partitions)
                yAf = roll.tile([D + 1, 512], F32, name="yAf", tag="yAf")
                yBf = roll.tile([D + 1, 512], F32, name="yBf", tag="yBf")
                nc.vector.tensor_copy(out=yAf, in_=yA)
                nc.vector.tensor_copy(out=yBf, in_=yB)
                nc.vector.reciprocal(out=yAf[D:D + 1, :], in_=yAf[D:D + 1, :])
                nc.vector.reciprocal(out=yBf[D:D + 1, :], in_=yBf[D:D + 1, :])
                rc_dram = dpool.tile([2, 512], F32, name="rc_dram",
                                     tag="rc_dram")
                nc.sync.dma_start(rc_dram[0:1, :], yAf[D:D + 1, :])
                nc.sync.dma_start(rc_dram[1:2, :], yBf[D:D + 1, :])
                pull(2)
                bc = roll.tile([D, 2, 512], F32, name="bc", tag="bc")
                nc.sync.dma_start(
                    bc, bass.AP(tensor=rc_dram.tensor, offset=rc_dram.offset,
                                ap=[[0, D], [512, 2], [1, 512]]))
                nc.vector.tensor_tensor(
                    out=yT_bf[0:D, p, :], in0=yAf[0:D, :], in1=bc[:, 0, :],
                    op=MUL)
                stB = roll.tile([D, 512], BF, name="stB", tag="stB")
                nc.vector.tensor_tensor(
                    out=stB, in0=yBf[0:D, :], in1=bc[:, 1, :], op=MUL)
                nc.sync.dma_start(out=yT_bf[D:P, p, :], in_=stB)
                pull(2)


        _SENT = object()

        # prologue: x DMAs first, then constants, then x pipeline of chunk 0
        # interleaved with weight piece 0, then chunk-0 qkT/kv matmuls
        # chasing the remaining weight pieces
        p00 = emit_p0(0)
        next(p00, None)     # issue the 4 x DMAs of chunk 0
        setup_consts()
        for _ in emit_w_loads(0):
            next(p00, None)
            next(p00, None)
        for _ in p00:
            pass
        g = emit_a1(0, range(TC))
        for _ in emit_w_loads(1):
            next(g, None)
        for _ in g:
            pass
        g = emit_a1(0, range(TC, 2 * TC))
        for _ in emit_w_loads(2):
            next(g, None)
        for _ in g:
            pass
        g = emit_a2(0)
        for _ in emit_w_loads(3):
            next(g, None)
        for _ in g:
            pass

        for tch in range(TCH):
            parts = []
            n_units = 0
            if tch >= 1:
                parts.append(emit_c(tch - 1))
                n_units += 20
            if tch + 1 < TCH:
                parts.append(emit_p0(tch + 1))
                parts.append(emit_a1(tch + 1, range(2 * TC)))
                parts.append(emit_a2(tch + 1))
                n_units += 17 + 24 + 32
            feeder = chain(*parts)
            emit_b(tch, feeder, n_units)
            for _ in feeder:   # drain any remaining filler work
                pass
        for _ in emit_c(TCH - 1):
            pass


_NC_CACHE = None


def _get_nc():
    global _NC_CACHE
    if _NC_CACHE is None:
        _NC_CACHE = build_nc()
    return _NC_CACHE


def _run(inputs, trace=False):
    nc = _get_nc()
    x = np.ascontiguousarray(np.asarray(inputs["x"], dtype=np.float32))
    w_attn = np.ascontiguousarray(np.asarray(inputs["w_attn"], dtype=np.float32))
    b_attn = np.ascontiguousarray(np.asarray(inputs["b_attn"], dtype=np.float32))
    w_proj = np.ascontiguousarray(np.asarray(inputs["w_proj"], dtype=np.float32))
    b_proj = np.ascontiguousarray(np.asarray(inputs["b_proj"], dtype=np.float32))
    in_maps = [
        {"x": x[i], "w_attn": w_attn, "b_attn": b_attn,
         "w_proj": w_proj, "b_proj": b_proj}
        for i in range(B)
    ]
    res = run_bass_kernel_spmd(nc, in_maps, core_ids=list(range(B)),
                               trace=trace)
    yo = np.stack([r["y"] for r in res.results])
    ko = np.stack([r["k_out"] for r in res.results])
    vo = np.stack([r["v_out"] for r in res.results])
    return (yo, ko, vo), res


def kernel(**inputs):
    outs, _ = _run(inputs, trace=False)
    return outs


def kernel_traced(**inputs):
    outs, res = _run(inputs, trace=True)
    return outs, res
